# revision 29
# baseline (speedup 1.0000x reference)
"""Causal self-attention with RoPE on 8 TRN2 NeuronCores — v3.

Problem: B=4, T=2048, D=1024, 16 heads x 64 dims, fp32, causal, RoPE.

Sharding: (batch b, head-group g) -> core b*2+g. Each core computes the
full sequence for 8 heads of one batch plus that group's partial output
projection; the host sums the two partial projections per batch.

v3 changes over v2:
  - fine-grained interleaving: projection / output-projection / normalize
    work is queued as small closures and pumped between attention j-tile
    steps, so the PE stays fed while the ACT engine grinds through exp
    (the attention inner loop is exp-bound at ~1.1us per j-tile vs
    ~0.64us of PE work)
  - reciprocal on ACT (Ln + Exp(-x), same natural_log_exp table set)
    instead of DVE's ~3.3us RECIPROCAL
  - initial weight DMAs split so the first rope matmul only waits for
    x chunk 0 + wq pair 0 (~1.3MB) instead of all weights
"""

import numpy as np
import ml_dtypes

import concourse.bass as bass
import concourse.tile as tile
import concourse.mybir as mybir

F32 = mybir.dt.float32
BF16 = mybir.dt.bfloat16

B, T, D = 4, 2048, 1024
NUM_HEADS, HEAD_DIM = 16, 64
ROPE_THETA = 10000.0

G = 512          # head dims per core (8 heads)
HPC = 8          # heads per core
PAIRS = 4        # pair-tiles (2 heads / 128 partitions)
KT = D // 128    # k-tiles over D
TC = 512         # i-chunk width
NCHUNK = T // TC
TT = T // 128    # t-tiles
N_CORES = 8

DT = BF16


def _split_multi_waits(nc, max_waits=1):
    """This walrus build rejects >1 sync-wait per instruction; spill extras
    onto same-engine NoOps placed just before."""
    counter = [0]
    for func in nc.m.functions:
        for bb in func.blocks:
            insts = bb.instructions
            if not any(
                ins.sync_info is not None and len(ins.sync_info.on_wait) > max_waits
                for ins in insts
            ):
                continue
            new_list = []
            for ins in insts:
                si = ins.sync_info
                if si is None or len(si.on_wait) <= max_waits:
                    new_list.append(ins)
                    continue
                waits = list(si.on_wait)
                spill, keep = waits[:-max_waits], waits[-max_waits:]
                for w in spill:
                    counter[0] += 1
                    new_list.append(
                        mybir.InstNoOp(
                            name=f"waitnop-{counter[0]}",
                            engine=ins.engine,
                            ins=[],
                            outs=[],
                            sync_info=mybir.SyncInfo(on_wait=[w], on_update=[]),
                        )
                    )
                ins.sync_info = mybir.SyncInfo(on_wait=keep, on_update=list(si.on_update))
                new_list.append(ins)
            bb.instructions = new_list


OPTS = {
    "recip": "act",      # act | dve
    "tail_recip": "act",
    "es_bufs": 8,
    "btmp_bufs": 4,
    "attu_bufs": 5,
    "mm_item": 2,        # proj matmuls per work item
}


def build_kernel():
    nc = bass.Bass()

    xT = nc.dram_tensor("xT", [D, T], DT, kind="ExternalInput")
    wqT = nc.dram_tensor("wqT", [D, G], DT, kind="ExternalInput")
    wkT = nc.dram_tensor("wkT", [D, G], DT, kind="ExternalInput")
    wvT = nc.dram_tensor("wvT", [D, G], DT, kind="ExternalInput")
    woT = nc.dram_tensor("woT", [G, D], DT, kind="ExternalInput")
    cos2 = nc.dram_tensor("cos2", [128, T], DT, kind="ExternalInput")
    sinsw = nc.dram_tensor("sinsw", [128, T], DT, kind="ExternalInput")
    tri01 = nc.dram_tensor("tri01", [128, 2, 128], DT, kind="ExternalInput")
    out = nc.dram_tensor("out", [T, D], F32, kind="ExternalOutput")

    LN = mybir.ActivationFunctionType.Ln
    EXP = mybir.ActivationFunctionType.Exp

    with tile.TileContext(nc) as tc:
        with (
            tc.tile_pool(name="const", bufs=1) as cpool,
            tc.tile_pool(name="qk", bufs=1) as qkpool,
            tc.tile_pool(name="vext", bufs=1) as vpool,
            tc.tile_pool(name="attn", bufs=1) as apool,
            tc.tile_pool(name="rope", bufs=2) as rpool,
            tc.tile_pool(name="exps", bufs=8) as epool,
            tc.tile_pool(name="norm", bufs=2) as npool,
            tc.tile_pool(name="outp", bufs=2) as opool,
            tc.tile_pool(name="dramb", bufs=4, space="DRAM") as dpool,
            tc.tile_pool(name="mm", bufs=2, space="PSUM") as mmps,
            tc.tile_pool(name="sp", bufs=2, space="PSUM") as spps,
            tc.tile_pool(name="ap", bufs=1, space="PSUM") as apps,
        ):
            xT_r = xT.rearrange("(k p) t -> p k t", p=128)

            # ---- staged initial DMAs: only what the first rope needs up
            # front; remaining weights stream in behind ----
            xc0 = rpool.tile([128, KT, TC], DT, name="xc0", tag="xc")
            wq_sb = cpool.tile([128, KT, G], DT, name="wq_sb")
            wqT_r = wqT.rearrange("(k p) g -> p k g", p=128)
            nc.sync.dma_start(xc0[:, 0:2, :], xT_r[:, 0:2, bass.ts(0, TC)])
            nc.sync.dma_start(wq_sb[:, 0:2, 0:128], wqT_r[:, 0:2, 0:128])
            nc.sync.dma_start(xc0[:, 2:4, :], xT_r[:, 2:4, bass.ts(0, TC)])
            nc.sync.dma_start(wq_sb[:, 2:4, 0:128], wqT_r[:, 2:4, 0:128])
            nc.sync.dma_start(xc0[:, 4:KT, :], xT_r[:, 4:KT, bass.ts(0, TC)])
            nc.sync.dma_start(wq_sb[:, 4:KT, 0:128], wqT_r[:, 4:KT, 0:128])
            cos_sb = cpool.tile([128, T], DT, name="cos_sb")
            sinsw_sb = cpool.tile([128, T], DT, name="sinsw_sb")
            nc.sync.dma_start(sinsw_sb[:, 0:TC], sinsw[:, 0:TC])
            nc.sync.dma_start(cos_sb[:, 0:TC], cos2[:, 0:TC])
            wk_sb = cpool.tile([128, KT, G], DT, name="wk_sb")
            wkT_r = wkT.rearrange("(k p) g -> p k g", p=128)
            nc.sync.dma_start(wk_sb[:, :, 0:128], wkT_r[:, :, 0:128])
            wv_sb = cpool.tile([128, KT, G], DT, name="wv_sb")
            nc.sync.dma_start(wv_sb[:], wvT.rearrange("(k p) g -> p k g", p=128))
            tri_sb = cpool.tile([128, 2, 128], DT, name="tri_sb")
            nc.sync.dma_start(tri_sb[:], tri01[:])
            wo_sb = cpool.tile([128, PAIRS, D], DT, name="wo_sb")
            ones_sb = cpool.tile([128, 64], DT, name="ones_sb")
            nc.vector.memset(ones_sb[:], 1.0)

            def rest_dmas_a():
                # issued after the pair-0 rope swaps so they don't delay them
                nc.sync.dma_start(wq_sb[:, :, 128:G], wqT_r[:, :, 128:G])
                nc.sync.dma_start(wk_sb[:, :, 128:G], wkT_r[:, :, 128:G])

            def rest_dmas_b():
                nc.sync.dma_start(sinsw_sb[:, TC:T], sinsw[:, TC:T])
                nc.sync.dma_start(cos_sb[:, TC:T], cos2[:, TC:T])

            def rest_dmas_c():
                nc.sync.dma_start(
                    wo_sb[:], woT.rearrange("(k p) d -> p k d", p=128)
                )

            qrot = qkpool.tile([128, PAIRS, T], DT, name="qrot")
            krot = qkpool.tile([128, PAIRS, T], DT, name="krot")
            v_ext = vpool.tile([128, TT, HPC, 65], DT, name="v_ext")
            attnT = apool.tile([128, PAIRS, T], DT, name="attnT")

            pend = {}
            xcs = {0: xc0}

            WORK = []

            def pump(n):
                for _ in range(n):
                    if not WORK:
                        return
                    WORK.pop(0)()

            def drain():
                while WORK:
                    WORK.pop(0)()

            # ---------- projection work items ----------

            def rope_unit_items(c, p, w_sb, rot):
                csl = bass.ts(c, TC)
                st = {}

                def mk_mm(k0, k1):
                    def f():
                        if "ps" not in st:
                            st["ps"] = mmps.tile(
                                [128, TC], F32, name="proj_ps", tag="mmps"
                            )
                        ps = st["ps"]
                        xc = xcs[c]
                        for k in range(k0, k1):
                            nc.tensor.matmul(
                                ps[:],
                                w_sb[:, k, bass.ts(p, 128)],
                                xc[:, k, :],
                                start=(k == 0),
                                stop=(k == KT - 1),
                            )
                    return f

                def dve():
                    ps = st["ps"]
                    u = rpool.tile([128, TC], DT, name="u", tag="u")
                    nc.vector.tensor_mul(u[:], ps[:], sinsw_sb[:, csl])
                    sw = rpool.tile([128, TC], DT, name="sw", tag="sw")
                    for blk in range(4):
                        sc = (blk ^ 1) * 32
                        nc.sync.dma_start(
                            sw[blk * 32 : blk * 32 + 32, :], u[sc : sc + 32, :]
                        )
                    t2 = rpool.tile([128, TC], DT, name="t2", tag="t2")
                    nc.vector.tensor_mul(t2[:], ps[:], cos_sb[:, csl])
                    nc.vector.tensor_add(rot[:, p, csl], t2[:], sw[:])

                step = OPTS["mm_item"]
                items = [mk_mm(k, min(k + step, KT)) for k in range(0, KT, step)]
                items.append(dve)
                return items

            def projv_unit_items(c, tt):
                t = 4 * c + tt
                st = {}

                def mk_mm(k0, k1):
                    def f():
                        if "ps" not in st:
                            st["ps"] = mmps.tile(
                                [128, G], F32, name="v_ps", tag="mmps"
                            )
                        ps = st["ps"]
                        xc = xcs[c]
                        for k in range(k0, k1):
                            nc.tensor.matmul(
                                ps[:],
                                xc[:, k, bass.ts(tt, 128)],
                                wv_sb[:, k, :],
                                start=(k == 0),
                                stop=(k == KT - 1),
                            )
                    return f

                def evac():
                    ps = st["ps"]
                    nc.vector.tensor_copy(
                        v_ext[:, t, :, 0:64],
                        ps[:].rearrange("p (h d) -> p h d", h=HPC),
                    )
                    nc.vector.memset(v_ext[:, t, :, 64:65], 1.0)

                step = OPTS["mm_item"]
                items = [mk_mm(k, min(k + step, KT)) for k in range(0, KT, step)]
                items.append(evac)
                return items

            def proj_items(c):
                items = []

                def xdma():
                    xcn = rpool.tile([128, KT, TC], DT, name="xcn", tag="xc")
                    nc.sync.dma_start(xcn[:], xT_r[:, :, bass.ts(c, TC)])
                    xcs[c] = xcn

                if c not in xcs:
                    items.append(xdma)
                items += rope_unit_items(c, 0, wq_sb, qrot)
                items += rope_unit_items(c, 0, wk_sb, krot)
                if c == 0:
                    items.append(rest_dmas_a)
                for tt in range(4):
                    items += projv_unit_items(c, tt)
                if c == 0:
                    items.append(rest_dmas_b)
                for p in range(1, PAIRS):
                    items += rope_unit_items(c, p, wq_sb, qrot)
                    items += rope_unit_items(c, p, wk_sb, krot)
                if c == 0:
                    items.append(rest_dmas_c)
                return items

            # ---------- output projection items ----------

            def outproj_items(c):
                items = []
                for tt in range(4):
                    t = 4 * c + tt
                    tsl = bass.ts(t, 128)
                    st = {}

                    def mk_dc(dc, st=st, tsl=tsl, t=t):
                        def f():
                            if "ob" not in st:
                                st["ob"] = opool.tile(
                                    [128, D], F32, name="ob", tag="ob"
                                )
                            dsl = bass.ts(dc, 512)
                            ps = mmps.tile([128, 512], F32, name="o_ps", tag="mmps")
                            for p in range(PAIRS):
                                nc.tensor.matmul(
                                    ps[:],
                                    attnT[:, p, tsl],
                                    wo_sb[:, p, dsl],
                                    start=(p == 0),
                                    stop=(p == PAIRS - 1),
                                )
                            nc.vector.tensor_copy(st["ob"][:, dsl], ps[:])
                            nc.sync.dma_start(
                                out[t * 128 : t * 128 + 128, dsl], st["ob"][:, dsl]
                            )
                        return f

                    items += [mk_dc(0), mk_dc(1)]
                return items

            # ---------- softmax normalization ----------

            def recip_rows(den_sb, rden, r0, r1, engine=None):
                if (engine or OPTS["recip"]) == "act":
                    nc.scalar.activation(den_sb[r0:r1, :], den_sb[r0:r1, :], LN)
                    nc.scalar.activation(
                        rden[r0:r1, :], den_sb[r0:r1, :], EXP, scale=-1.0
                    )
                else:
                    with nc.allow_low_precision(reason="bf16 recip rows"):
                        nc.vector.reciprocal(rden[r0:r1, :], den_sb[r0:r1, :])

            def bounce(rden, r0, nrows):
                dscr = dpool.tile([nrows, TC], F32, name="dscr", tag=f"dscr{nrows}")
                nc.sync.dma_start(dscr[:], rden[r0 : r0 + nrows, :])
                rbc = npool.tile(
                    [64, nrows, TC], F32, name="rbc", tag=f"rbc{nrows}", bufs=2
                )
                dsrc = dscr[:]
                nc.sync.dma_start(
                    rbc[:],
                    bass.AP(
                        tensor=dsrc.tensor,
                        offset=dsrc.offset,
                        ap=[[0, 64]] + dsrc.ap,
                    ),
                )
                return rbc

            def norm_muls(c, p, attU, rbc, rcol):
                csl = bass.ts(c, TC)
                nc.vector.tensor_mul(
                    attnT[0:64, p, csl], attU[0:64, 0, :], rbc[:, rcol, :]
                )
                btmp = npool.tile(
                    [64, TC], DT, name="btmp", tag="btmp", bufs=OPTS["btmp_bufs"]
                )
                nc.vector.tensor_mul(btmp[:], attU[0:64, 1, :], rbc[:, rcol + 1, :])
                nc.sync.dma_start(attnT[64:128, p, csl], btmp[:])

            def normalize_items(c):
                items = []

                def recip():
                    attUs, den_sb, rden = pend[c]
                    recip_rows(den_sb, rden, 0, 8)

                items.append(recip)
                for p in range(PAIRS):

                    def mk_np(p=p):
                        def f():
                            attUs, den_sb, rden = pend[c]
                            rbc = bounce(rden, 2 * p, 2)
                            norm_muls(c, p, attUs[p], rbc, 0)
                        return f

                    items.append(mk_np())
                items.append(lambda: pend.pop(c))
                return items

            # ---------- attention ----------

            def attn_pair(c, p, den_sb, dr0, steps_left, last=False):
                atts = apps.tile([128, 2, TC], F32, name="att_ps", tag="apps")
                njt = 4 * c + 4

                def av(jt, es, off, fd):
                    for hh in range(2):
                        nc.tensor.matmul(
                            atts[0:65, hh, off : off + fd],
                            v_ext[:, jt, 2 * p + hh, :],
                            es[:, hh, 0:fd],
                            start=(jt == 0),
                            stop=(jt == njt - 1),
                        )

                for jt in range(njt):
                    m = jt - 4 * c
                    sAB = spps.tile([128, 2, TC], F32, name="s_ps", tag="spps")
                    soff = 128 * m if m > 0 else 0
                    for hh in range(2):
                        hsl = slice(64 * hh, 64 * hh + 64)
                        nc.tensor.matmul(
                            sAB[:, hh, soff:TC],
                            krot[hsl, p, bass.ts(jt, 128)],
                            qrot[hsl, p, c * TC + soff : (c + 1) * TC],
                            start=True,
                            stop=True,
                            tile_position=(64 * hh, 0),
                        )
                    off = 128 * m if m > 0 else 0
                    fd = TC - off
                    es = epool.tile(
                        [128, 2, TC], DT, name="es", tag="es", bufs=OPTS["es_bufs"]
                    )
                    nc.scalar.activation(
                        es[:, :, 0:fd],
                        sAB[:, :, off : off + fd],
                        EXP,
                        scale=0.125,
                    )
                    if m >= 0:
                        # last chunk: DVE queue carries the pumped outproj
                        # evacuations; park the mask on the idle GpSimd
                        eng = nc.gpsimd if c == NCHUNK - 1 else nc.vector
                        eng.tensor_mul(
                            es[:, :, 0:128], es[:, :, 0:128], tri_sb[:]
                        )
                    av(jt, es, off, fd)
                    # feed the PE between exp-bound steps
                    sl = steps_left[0]
                    if WORK and sl > 0:
                        r = -(-len(WORK) // sl)
                        pump(min(r, 6))
                    steps_left[0] -= 1
                attU = npool.tile(
                    [128, 2, TC], F32, name="attU", tag="attU",
                    bufs=OPTS["attu_bufs"],
                )
                if last:
                    # sliced tail evacuates per t-slice itself
                    return attU, atts
                # per-head evacuation: the next pair's first AV (start=True)
                # only waits for its own bank's reader
                for hh in range(2):
                    nc.vector.tensor_copy(attU[0:65, hh, :], atts[0:65, hh, :])
                    nc.sync.dma_start(
                        den_sb[dr0 + hh : dr0 + hh + 1, :], attU[64:65, hh, :]
                    )
                return attU, atts

            def tail_norm_pair(c, p, attU, atts, rdrow, sliced):
                """Last-chunk normalize without DRAM bounces: reciprocal on ACT
                straight from the AV psum's denominator row, broadcast across
                partitions with a K=1 matmul, multiply. When `sliced`, process
                128-wide t-slices and chase each with its output projection to
                keep the PE warm through the tail."""
                c0 = c * TC
                nslice = 4 if sliced else 1
                w = TC // nslice
                rbc = spps.tile([128, 2, TC], F32, name="rbc_ps", tag="spps")
                for s in range(nslice):
                    ssl = slice(s * w, s * w + w)
                    for hh in range(2):
                        nc.vector.tensor_copy(
                            attU[0:65, hh, ssl], atts[0:65, hh, ssl]
                        )
                    nc.scalar.activation(
                        attU[64:65, :, ssl], attU[64:65, :, ssl], LN
                    )
                    nc.scalar.activation(
                        rdrow[64:65, :, ssl], attU[64:65, :, ssl], EXP, scale=-1.0
                    )
                    for hh in range(2):
                        nc.tensor.matmul(
                            rbc[0:64, hh, ssl],
                            ones_sb[64:65, :],
                            rdrow[64:65, hh, ssl],
                            start=True,
                            stop=True,
                        )
                    btmp = npool.tile(
                        [64, TC], DT, name="btmp", tag="btmp",
                        bufs=OPTS["btmp_bufs"],
                    )
                    nc.vector.tensor_mul(
                        btmp[:, ssl], attU[0:64, 1, ssl], rbc[0:64, 1, ssl]
                    )
                    nc.sync.dma_start(
                        attnT[64:128, p, c0 + s * w : c0 + (s + 1) * w],
                        btmp[:, ssl],
                    )
                    nc.vector.tensor_mul(
                        attnT[0:64, p, c0 + s * w : c0 + (s + 1) * w],
                        attU[0:64, 0, ssl],
                        rbc[0:64, 0, ssl],
                    )
                    if sliced and s > 0:
                        # project the PREVIOUS slice's t-tile: its btmp DMA
                        # completed while this slice's recip ran
                        outproj_tile(4 * c + s - 1)
                if sliced:
                    outproj_tile(4 * c + nslice - 1)

            def outproj_tile(t):
                tsl = bass.ts(t, 128)
                ob = opool.tile([128, D], F32, name="ob", tag="ob")
                for dc in range(2):
                    dsl = bass.ts(dc, 512)
                    ps = mmps.tile([128, 512], F32, name="o_ps", tag="mmps")
                    for pp in range(PAIRS):
                        nc.tensor.matmul(
                            ps[:],
                            attnT[:, pp, tsl],
                            wo_sb[:, pp, dsl],
                            start=(pp == 0),
                            stop=(pp == PAIRS - 1),
                        )
                    nc.vector.tensor_copy(ob[:, dsl], ps[:])
                    nc.sync.dma_start(
                        out[t * 128 : t * 128 + 128, dsl], ob[:, dsl]
                    )

            def attn_chunk(c):
                last = c == NCHUNK - 1
                den_sb = npool.tile([128, TC], F32, name="den_sb", tag="den")
                rden = npool.tile([128, TC], F32, name="rden", tag="rden")
                attUs = []
                steps_left = [PAIRS * (4 * c + 4)]
                for p in range(PAIRS):
                    tail_sliced = last and p == PAIRS - 1
                    dr0 = 32 * p if last else 2 * p
                    attU, atts = attn_pair(
                        c, p, den_sb, dr0, steps_left, last=tail_sliced
                    )
                    if tail_sliced:
                        rdrow = npool.tile(
                            [65, 2, TC], DT, name="rdrow", tag="rdrow", bufs=1
                        )
                        tail_norm_pair(c, p, attU, atts, rdrow, sliced=True)
                    elif last:
                        # earlier tail pairs: normalize per-pair off the PE
                        # path (den rows -> ACT recip -> DRAM-bounce bcast)
                        recip_rows(den_sb, rden, 32 * p, 32 * p + 2)
                        rbc = bounce(rden, 32 * p, 2)
                        norm_muls(c, p, attU, rbc, 0)
                    else:
                        attUs.append(attU)
                if not last:
                    pend[c] = (attUs, den_sb, rden)

            # ---------- main schedule ----------

            for it in proj_items(0):
                it()
            for c in range(NCHUNK):
                if c > 0:
                    WORK.extend(normalize_items(c - 1))
                if c + 1 < NCHUNK:
                    WORK.extend(proj_items(c + 1))
                # ALL output projections are deferred into the last chunk's
                # attention window, which is otherwise exp-bound with an
                # idle PE; earlier windows are already PE-bound
                if c == NCHUNK - 1:
                    for cc in range(NCHUNK - 1):
                        WORK.extend(outproj_items(cc))
                attn_chunk(c)
                drain()

    _split_multi_waits(nc)
    return nc


def _to_dt(x):
    return np.ascontiguousarray(x).astype(ml_dtypes.bfloat16)


def _rope_tables():
    inv_freq = 1.0 / ROPE_THETA ** (np.arange(0, HEAD_DIM, 2, dtype=np.float64) / HEAD_DIM)
    freqs = np.outer(np.arange(T, dtype=np.float64), inv_freq)  # [T, 32]
    cos_t = np.cos(freqs).T.astype(np.float32)  # [32, T]
    sin_t = np.sin(freqs).T.astype(np.float32)
    cos2 = np.concatenate([cos_t] * 4, axis=0)  # [128, T]
    sin2 = np.concatenate([sin_t] * 4, axis=0)
    sgn = np.ones((128, 1), np.float32)
    sgn[0:32] = -1.0
    sgn[64:96] = -1.0
    sinsw = -sgn * sin2
    return _to_dt(cos2), _to_dt(sinsw)


def _tri01():
    j = np.arange(128)[:, None]
    c = np.arange(128)[None, :]
    t = np.where(j <= c, 1.0, 0.0).astype(np.float32)  # [128, 128]
    return _to_dt(np.repeat(t[:, None, :], 2, axis=1))  # [128, 2, 128]


_NC_CACHE = {}
LAST_RESULTS = None  # BassKernelResults of the most recent kernel() call


def kernel(x, wq, wk, wv, wo):
    global LAST_RESULTS
    from concourse.bass_utils import run_bass_kernel_spmd

    x = np.asarray(x, dtype=np.float32)
    wq = np.asarray(wq, dtype=np.float32)
    wk = np.asarray(wk, dtype=np.float32)
    wv = np.asarray(wv, dtype=np.float32)
    wo = np.asarray(wo, dtype=np.float32)

    cos2, sinsw = _rope_tables()
    tri = _tri01()

    in_maps = []
    for core in range(N_CORES):
        b, g = core // 2, core % 2
        gs = slice(G * g, G * g + G)
        in_maps.append(
            {
                "xT": _to_dt(x[b].T),
                "wqT": _to_dt(wq[gs].T),
                "wkT": _to_dt(wk[gs].T),
                "wvT": _to_dt(wv[gs].T),
                "woT": _to_dt(wo[:, gs].T),
                "cos2": cos2,
                "sinsw": sinsw,
                "tri01": tri,
            }
        )

    if "nc" not in _NC_CACHE:
        _NC_CACHE["nc"] = build_kernel()
    nc = _NC_CACHE["nc"]

    res = run_bass_kernel_spmd(nc, in_maps, core_ids=list(range(N_CORES)))
    LAST_RESULTS = res
    outs = [r["out"] for r in res.results]
    full = np.empty((B, T, D), dtype=np.float32)
    for b in range(B):
        full[b] = (
            outs[2 * b].astype(np.float64) + outs[2 * b + 1].astype(np.float64)
        ).astype(np.float32)
    return full


# revision 31
# speedup vs baseline: 1.0118x; 1.0118x over previous
"""Causal self-attention with RoPE on 8 TRN2 NeuronCores — v3.

Problem: B=4, T=2048, D=1024, 16 heads x 64 dims, fp32, causal, RoPE.

Sharding: (batch b, head-group g) -> core b*2+g. Each core computes the
full sequence for 8 heads of one batch plus that group's partial output
projection; the host sums the two partial projections per batch.

v3 changes over v2:
  - fine-grained interleaving: projection / output-projection / normalize
    work is queued as small closures and pumped between attention j-tile
    steps, so the PE stays fed while the ACT engine grinds through exp
    (the attention inner loop is exp-bound at ~1.1us per j-tile vs
    ~0.64us of PE work)
  - reciprocal on ACT (Ln + Exp(-x), same natural_log_exp table set)
    instead of DVE's ~3.3us RECIPROCAL
  - initial weight DMAs split so the first rope matmul only waits for
    x chunk 0 + wq pair 0 (~1.3MB) instead of all weights
"""

import numpy as np
import ml_dtypes

import concourse.bass as bass
import concourse.tile as tile
import concourse.mybir as mybir

F32 = mybir.dt.float32
BF16 = mybir.dt.bfloat16

B, T, D = 4, 2048, 1024
NUM_HEADS, HEAD_DIM = 16, 64
ROPE_THETA = 10000.0

G = 512          # head dims per core (8 heads)
HPC = 8          # heads per core
PAIRS = 4        # pair-tiles (2 heads / 128 partitions)
KT = D // 128    # k-tiles over D
TC = 512         # i-chunk width
NCHUNK = T // TC
TT = T // 128    # t-tiles
N_CORES = 8

DT = BF16


def _split_multi_waits(nc, max_waits=1):
    """This walrus build rejects >1 sync-wait per instruction; spill extras
    onto same-engine NoOps placed just before."""
    counter = [0]
    for func in nc.m.functions:
        for bb in func.blocks:
            insts = bb.instructions
            if not any(
                ins.sync_info is not None and len(ins.sync_info.on_wait) > max_waits
                for ins in insts
            ):
                continue
            new_list = []
            for ins in insts:
                si = ins.sync_info
                if si is None or len(si.on_wait) <= max_waits:
                    new_list.append(ins)
                    continue
                waits = list(si.on_wait)
                spill, keep = waits[:-max_waits], waits[-max_waits:]
                for w in spill:
                    counter[0] += 1
                    new_list.append(
                        mybir.InstNoOp(
                            name=f"waitnop-{counter[0]}",
                            engine=ins.engine,
                            ins=[],
                            outs=[],
                            sync_info=mybir.SyncInfo(on_wait=[w], on_update=[]),
                        )
                    )
                ins.sync_info = mybir.SyncInfo(on_wait=keep, on_update=list(si.on_update))
                new_list.append(ins)
            bb.instructions = new_list


OPTS = {
    "recip": "act",      # act | dve
    "tail_recip": "act",
    "es_bufs": 10,
    "btmp_bufs": 4,
    "attu_bufs": 6,
    "mm_item": 2,        # proj matmuls per work item
}


def build_kernel():
    nc = bass.Bass()

    xT = nc.dram_tensor("xT", [D, T], DT, kind="ExternalInput")
    wqT = nc.dram_tensor("wqT", [D, G], DT, kind="ExternalInput")
    wkT = nc.dram_tensor("wkT", [D, G], DT, kind="ExternalInput")
    wvT = nc.dram_tensor("wvT", [D, G], DT, kind="ExternalInput")
    woT = nc.dram_tensor("woT", [G, D], DT, kind="ExternalInput")
    cos2 = nc.dram_tensor("cos2", [128, T], DT, kind="ExternalInput")
    sinsw = nc.dram_tensor("sinsw", [128, T], DT, kind="ExternalInput")
    tri01 = nc.dram_tensor("tri01", [128, 2, 128], DT, kind="ExternalInput")
    out = nc.dram_tensor("out", [T, D], F32, kind="ExternalOutput")

    LN = mybir.ActivationFunctionType.Ln
    EXP = mybir.ActivationFunctionType.Exp

    with tile.TileContext(nc) as tc:
        with (
            tc.tile_pool(name="const", bufs=1) as cpool,
            tc.tile_pool(name="qk", bufs=1) as qkpool,
            tc.tile_pool(name="vext", bufs=1) as vpool,
            tc.tile_pool(name="attn", bufs=1) as apool,
            tc.tile_pool(name="rope", bufs=2) as rpool,
            tc.tile_pool(name="exps", bufs=8) as epool,
            tc.tile_pool(name="norm", bufs=2) as npool,
            tc.tile_pool(name="outp", bufs=2) as opool,
            tc.tile_pool(name="dramb", bufs=4, space="DRAM") as dpool,
            tc.tile_pool(name="mm", bufs=2, space="PSUM") as mmps,
            tc.tile_pool(name="sp", bufs=2, space="PSUM") as spps,
            tc.tile_pool(name="ap", bufs=1, space="PSUM") as apps,
        ):
            xT_r = xT.rearrange("(k p) t -> p k t", p=128)

            # ---- staged initial DMAs: only what the first rope needs up
            # front; remaining weights stream in behind ----
            xc0 = rpool.tile([128, KT, TC], DT, name="xc0", tag="xc")
            wq_sb = cpool.tile([128, KT, G], DT, name="wq_sb")
            wqT_r = wqT.rearrange("(k p) g -> p k g", p=128)
            nc.sync.dma_start(xc0[:, 0:2, :], xT_r[:, 0:2, bass.ts(0, TC)])
            nc.sync.dma_start(wq_sb[:, 0:2, 0:128], wqT_r[:, 0:2, 0:128])
            nc.sync.dma_start(xc0[:, 2:4, :], xT_r[:, 2:4, bass.ts(0, TC)])
            nc.sync.dma_start(wq_sb[:, 2:4, 0:128], wqT_r[:, 2:4, 0:128])
            nc.sync.dma_start(xc0[:, 4:KT, :], xT_r[:, 4:KT, bass.ts(0, TC)])
            nc.sync.dma_start(wq_sb[:, 4:KT, 0:128], wqT_r[:, 4:KT, 0:128])
            cos_sb = cpool.tile([128, T], DT, name="cos_sb")
            sinsw_sb = cpool.tile([128, T], DT, name="sinsw_sb")
            nc.sync.dma_start(sinsw_sb[:, 0:TC], sinsw[:, 0:TC])
            nc.sync.dma_start(cos_sb[:, 0:TC], cos2[:, 0:TC])
            wk_sb = cpool.tile([128, KT, G], DT, name="wk_sb")
            wkT_r = wkT.rearrange("(k p) g -> p k g", p=128)
            nc.sync.dma_start(wk_sb[:, :, 0:128], wkT_r[:, :, 0:128])
            wv_sb = cpool.tile([128, KT, G], DT, name="wv_sb")
            nc.sync.dma_start(wv_sb[:], wvT.rearrange("(k p) g -> p k g", p=128))
            tri_sb = cpool.tile([128, 2, 128], DT, name="tri_sb")
            nc.sync.dma_start(tri_sb[:], tri01[:])
            wo_sb = cpool.tile([128, PAIRS, D], DT, name="wo_sb")
            ones_sb = cpool.tile([128, 64], DT, name="ones_sb")
            nc.vector.memset(ones_sb[:], 1.0)

            def rest_dmas_a():
                # issued after the pair-0 rope swaps so they don't delay them
                nc.sync.dma_start(wq_sb[:, :, 128:G], wqT_r[:, :, 128:G])
                nc.sync.dma_start(wk_sb[:, :, 128:G], wkT_r[:, :, 128:G])

            def rest_dmas_b():
                nc.sync.dma_start(sinsw_sb[:, TC:T], sinsw[:, TC:T])
                nc.sync.dma_start(cos_sb[:, TC:T], cos2[:, TC:T])

            def rest_dmas_c():
                nc.sync.dma_start(
                    wo_sb[:], woT.rearrange("(k p) d -> p k d", p=128)
                )

            qrot = qkpool.tile([128, PAIRS, T], DT, name="qrot")
            krot = qkpool.tile([128, PAIRS, T], DT, name="krot")
            v_ext = vpool.tile([128, TT, HPC, 65], DT, name="v_ext")
            attnT = apool.tile([128, PAIRS, T], DT, name="attnT")

            pend = {}
            xcs = {0: xc0}

            WORK = []

            def pump(n):
                for _ in range(n):
                    if not WORK:
                        return
                    WORK.pop(0)()

            def drain():
                while WORK:
                    WORK.pop(0)()

            # ---------- projection work items ----------

            def rope_unit_items(c, p, w_sb, rot):
                csl = bass.ts(c, TC)
                st = {}

                def mk_mm(k0, k1):
                    def f():
                        if "ps" not in st:
                            st["ps"] = mmps.tile(
                                [128, TC], F32, name="proj_ps", tag="mmps"
                            )
                        ps = st["ps"]
                        xc = xcs[c]
                        for k in range(k0, k1):
                            nc.tensor.matmul(
                                ps[:],
                                w_sb[:, k, bass.ts(p, 128)],
                                xc[:, k, :],
                                start=(k == 0),
                                stop=(k == KT - 1),
                            )
                    return f

                def dve():
                    ps = st["ps"]
                    u = rpool.tile([128, TC], DT, name="u", tag="u")
                    nc.vector.tensor_mul(u[:], ps[:], sinsw_sb[:, csl])
                    sw = rpool.tile([128, TC], DT, name="sw", tag="sw")
                    for blk in range(4):
                        sc = (blk ^ 1) * 32
                        nc.sync.dma_start(
                            sw[blk * 32 : blk * 32 + 32, :], u[sc : sc + 32, :]
                        )
                    t2 = rpool.tile([128, TC], DT, name="t2", tag="t2")
                    nc.vector.tensor_mul(t2[:], ps[:], cos_sb[:, csl])
                    nc.vector.tensor_add(rot[:, p, csl], t2[:], sw[:])

                step = OPTS["mm_item"]
                items = [mk_mm(k, min(k + step, KT)) for k in range(0, KT, step)]
                items.append(dve)
                return items

            def projv_unit_items(c, tt):
                t = 4 * c + tt
                st = {}

                def mk_mm(k0, k1):
                    def f():
                        if "ps" not in st:
                            st["ps"] = mmps.tile(
                                [128, G], F32, name="v_ps", tag="mmps"
                            )
                        ps = st["ps"]
                        xc = xcs[c]
                        for k in range(k0, k1):
                            nc.tensor.matmul(
                                ps[:],
                                xc[:, k, bass.ts(tt, 128)],
                                wv_sb[:, k, :],
                                start=(k == 0),
                                stop=(k == KT - 1),
                            )
                    return f

                def evac():
                    ps = st["ps"]
                    nc.vector.tensor_copy(
                        v_ext[:, t, :, 0:64],
                        ps[:].rearrange("p (h d) -> p h d", h=HPC),
                    )
                    nc.vector.memset(v_ext[:, t, :, 64:65], 1.0)

                step = OPTS["mm_item"]
                items = [mk_mm(k, min(k + step, KT)) for k in range(0, KT, step)]
                items.append(evac)
                return items

            def proj_items(c):
                items = []

                def xdma():
                    xcn = rpool.tile([128, KT, TC], DT, name="xcn", tag="xc")
                    nc.sync.dma_start(xcn[:], xT_r[:, :, bass.ts(c, TC)])
                    xcs[c] = xcn

                if c not in xcs:
                    items.append(xdma)
                items += rope_unit_items(c, 0, wq_sb, qrot)
                items += rope_unit_items(c, 0, wk_sb, krot)
                if c == 0:
                    items.append(rest_dmas_a)
                for tt in range(4):
                    items += projv_unit_items(c, tt)
                if c == 0:
                    items.append(rest_dmas_b)
                for p in range(1, PAIRS):
                    items += rope_unit_items(c, p, wq_sb, qrot)
                    items += rope_unit_items(c, p, wk_sb, krot)
                if c == 0:
                    items.append(rest_dmas_c)
                return items

            # ---------- output projection items ----------

            def outproj_items(c):
                items = []
                for tt in range(4):
                    t = 4 * c + tt
                    tsl = bass.ts(t, 128)
                    st = {}

                    def mk_dc(dc, st=st, tsl=tsl, t=t):
                        def f():
                            if "ob" not in st:
                                st["ob"] = opool.tile(
                                    [128, D], F32, name="ob", tag="ob"
                                )
                            dsl = bass.ts(dc, 512)
                            ps = mmps.tile([128, 512], F32, name="o_ps", tag="mmps")
                            for p in range(PAIRS):
                                nc.tensor.matmul(
                                    ps[:],
                                    attnT[:, p, tsl],
                                    wo_sb[:, p, dsl],
                                    start=(p == 0),
                                    stop=(p == PAIRS - 1),
                                )
                            nc.vector.tensor_copy(st["ob"][:, dsl], ps[:])
                            nc.sync.dma_start(
                                out[t * 128 : t * 128 + 128, dsl], st["ob"][:, dsl]
                            )
                        return f

                    items += [mk_dc(0), mk_dc(1)]
                return items

            # ---------- softmax normalization ----------

            def recip_rows(den_sb, rden, r0, r1, engine=None):
                if (engine or OPTS["recip"]) == "act":
                    nc.scalar.activation(den_sb[r0:r1, :], den_sb[r0:r1, :], LN)
                    nc.scalar.activation(
                        rden[r0:r1, :], den_sb[r0:r1, :], EXP, scale=-1.0
                    )
                else:
                    with nc.allow_low_precision(reason="bf16 recip rows"):
                        nc.vector.reciprocal(rden[r0:r1, :], den_sb[r0:r1, :])

            def bounce(rden, r0, nrows):
                dscr = dpool.tile([nrows, TC], F32, name="dscr", tag=f"dscr{nrows}")
                nc.sync.dma_start(dscr[:], rden[r0 : r0 + nrows, :])
                rbc = npool.tile(
                    [64, nrows, TC], F32, name="rbc", tag=f"rbc{nrows}", bufs=2
                )
                dsrc = dscr[:]
                nc.sync.dma_start(
                    rbc[:],
                    bass.AP(
                        tensor=dsrc.tensor,
                        offset=dsrc.offset,
                        ap=[[0, 64]] + dsrc.ap,
                    ),
                )
                return rbc

            def norm_muls(c, p, attU, rbc, rcol):
                csl = bass.ts(c, TC)
                nc.vector.tensor_mul(
                    attnT[0:64, p, csl], attU[0:64, 0, :], rbc[:, rcol, :]
                )
                btmp = npool.tile(
                    [64, TC], DT, name="btmp", tag="btmp", bufs=OPTS["btmp_bufs"]
                )
                nc.vector.tensor_mul(btmp[:], attU[0:64, 1, :], rbc[:, rcol + 1, :])
                nc.sync.dma_start(attnT[64:128, p, csl], btmp[:])

            def normalize_items(c):
                items = []

                def recip():
                    attUs, den_sb, rden = pend[c]
                    recip_rows(den_sb, rden, 0, 8)

                items.append(recip)
                for p in range(PAIRS):

                    def mk_np(p=p):
                        def f():
                            attUs, den_sb, rden = pend[c]
                            rbc = bounce(rden, 2 * p, 2)
                            norm_muls(c, p, attUs[p], rbc, 0)
                        return f

                    items.append(mk_np())
                items.append(lambda: pend.pop(c))
                return items

            # ---------- attention ----------

            def attn_pair(c, p, den_sb, dr0, steps_left, last=False):
                atts = apps.tile([128, 2, TC], F32, name="att_ps", tag="apps")
                njt = 4 * c + 4

                def av(jt, es, off, fd):
                    for hh in range(2):
                        nc.tensor.matmul(
                            atts[0:65, hh, off : off + fd],
                            v_ext[:, jt, 2 * p + hh, :],
                            es[:, hh, 0:fd],
                            start=(jt == 0),
                            stop=(jt == njt - 1),
                        )

                for jt in range(njt):
                    m = jt - 4 * c
                    sAB = spps.tile([128, 2, TC], F32, name="s_ps", tag="spps")
                    soff = 128 * m if m > 0 else 0
                    for hh in range(2):
                        hsl = slice(64 * hh, 64 * hh + 64)
                        nc.tensor.matmul(
                            sAB[:, hh, soff:TC],
                            krot[hsl, p, bass.ts(jt, 128)],
                            qrot[hsl, p, c * TC + soff : (c + 1) * TC],
                            start=True,
                            stop=True,
                            tile_position=(64 * hh, 0),
                        )
                    off = 128 * m if m > 0 else 0
                    fd = TC - off
                    es = epool.tile(
                        [128, 2, TC], DT, name="es", tag="es", bufs=OPTS["es_bufs"]
                    )
                    nc.scalar.activation(
                        es[:, :, 0:fd],
                        sAB[:, :, off : off + fd],
                        EXP,
                        scale=0.125,
                    )
                    if m >= 0:
                        nc.vector.tensor_mul(
                            es[:, :, 0:128], es[:, :, 0:128], tri_sb[:]
                        )
                    av(jt, es, off, fd)
                    # feed the PE between exp-bound steps
                    sl = steps_left[0]
                    if WORK and sl > 0:
                        r = -(-len(WORK) // sl)
                        pump(min(r, 6))
                    steps_left[0] -= 1
                attU = npool.tile(
                    [128, 2, TC], F32, name="attU", tag="attU",
                    bufs=OPTS["attu_bufs"],
                )
                if last:
                    # sliced tail evacuates per t-slice itself
                    return attU, atts
                # per-head evacuation: the next pair's first AV (start=True)
                # only waits for its own bank's reader
                for hh in range(2):
                    nc.vector.tensor_copy(attU[0:65, hh, :], atts[0:65, hh, :])
                    nc.sync.dma_start(
                        den_sb[dr0 + hh : dr0 + hh + 1, :], attU[64:65, hh, :]
                    )
                return attU, atts

            def tail_norm_pair(c, p, attU, atts, rdrow, sliced):
                """Last-chunk normalize without DRAM bounces: reciprocal on ACT
                straight from the AV psum's denominator row, broadcast across
                partitions with a K=1 matmul, multiply. When `sliced`, process
                128-wide t-slices and chase each with its output projection to
                keep the PE warm through the tail."""
                c0 = c * TC
                nslice = 4 if sliced else 1
                w = TC // nslice
                rbc = spps.tile([128, 2, TC], F32, name="rbc_ps", tag="spps")
                for s in range(nslice):
                    ssl = slice(s * w, s * w + w)
                    for hh in range(2):
                        nc.vector.tensor_copy(
                            attU[0:65, hh, ssl], atts[0:65, hh, ssl]
                        )
                    nc.scalar.activation(
                        attU[64:65, :, ssl], attU[64:65, :, ssl], LN
                    )
                    nc.scalar.activation(
                        rdrow[64:65, :, ssl], attU[64:65, :, ssl], EXP, scale=-1.0
                    )
                    for hh in range(2):
                        nc.tensor.matmul(
                            rbc[0:64, hh, ssl],
                            ones_sb[64:65, :],
                            rdrow[64:65, hh, ssl],
                            start=True,
                            stop=True,
                        )
                    btmp = npool.tile(
                        [64, TC], DT, name="btmp", tag="btmp",
                        bufs=OPTS["btmp_bufs"],
                    )
                    nc.vector.tensor_mul(
                        btmp[:, ssl], attU[0:64, 1, ssl], rbc[0:64, 1, ssl]
                    )
                    nc.sync.dma_start(
                        attnT[64:128, p, c0 + s * w : c0 + (s + 1) * w],
                        btmp[:, ssl],
                    )
                    nc.vector.tensor_mul(
                        attnT[0:64, p, c0 + s * w : c0 + (s + 1) * w],
                        attU[0:64, 0, ssl],
                        rbc[0:64, 0, ssl],
                    )
                    if sliced and s > 0:
                        # project the PREVIOUS slice's t-tile: its btmp DMA
                        # completed while this slice's recip ran
                        outproj_tile(4 * c + s - 1)
                if sliced:
                    outproj_tile(4 * c + nslice - 1)

            def outproj_tile(t):
                tsl = bass.ts(t, 128)
                ob = opool.tile([128, D], F32, name="ob", tag="ob")
                for dc in range(2):
                    dsl = bass.ts(dc, 512)
                    ps = mmps.tile([128, 512], F32, name="o_ps", tag="mmps")
                    for pp in range(PAIRS):
                        nc.tensor.matmul(
                            ps[:],
                            attnT[:, pp, tsl],
                            wo_sb[:, pp, dsl],
                            start=(pp == 0),
                            stop=(pp == PAIRS - 1),
                        )
                    nc.vector.tensor_copy(ob[:, dsl], ps[:])
                    nc.sync.dma_start(
                        out[t * 128 : t * 128 + 128, dsl], ob[:, dsl]
                    )

            def attn_chunk(c):
                last = c == NCHUNK - 1
                den_sb = npool.tile([128, TC], F32, name="den_sb", tag="den")
                rden = npool.tile([128, TC], F32, name="rden", tag="rden")
                attUs = []
                steps_left = [PAIRS * (4 * c + 4)]
                for p in range(PAIRS):
                    tail_sliced = last and p == PAIRS - 1
                    dr0 = 32 * p if last else 2 * p
                    attU, atts = attn_pair(
                        c, p, den_sb, dr0, steps_left, last=tail_sliced
                    )
                    if tail_sliced:
                        rdrow = npool.tile(
                            [65, 2, TC], DT, name="rdrow", tag="rdrow", bufs=1
                        )
                        tail_norm_pair(c, p, attU, atts, rdrow, sliced=True)
                    elif last:
                        # earlier tail pairs: normalize per-pair off the PE
                        # path (den rows -> ACT recip -> DRAM-bounce bcast)
                        recip_rows(den_sb, rden, 32 * p, 32 * p + 2)
                        rbc = bounce(rden, 32 * p, 2)
                        norm_muls(c, p, attU, rbc, 0)
                    else:
                        attUs.append(attU)
                if not last:
                    pend[c] = (attUs, den_sb, rden)

            # ---------- main schedule ----------

            for it in proj_items(0):
                it()
            for c in range(NCHUNK):
                if c > 0:
                    WORK.extend(normalize_items(c - 1))
                if c + 1 < NCHUNK:
                    WORK.extend(proj_items(c + 1))
                # ALL output projections are deferred into the last chunk's
                # attention window, which is otherwise exp-bound with an
                # idle PE; earlier windows are already PE-bound
                if c == NCHUNK - 1:
                    for cc in range(NCHUNK - 1):
                        WORK.extend(outproj_items(cc))
                attn_chunk(c)
                drain()

    _split_multi_waits(nc)
    return nc


def _to_dt(x):
    return np.ascontiguousarray(x).astype(ml_dtypes.bfloat16)


def _rope_tables():
    inv_freq = 1.0 / ROPE_THETA ** (np.arange(0, HEAD_DIM, 2, dtype=np.float64) / HEAD_DIM)
    freqs = np.outer(np.arange(T, dtype=np.float64), inv_freq)  # [T, 32]
    cos_t = np.cos(freqs).T.astype(np.float32)  # [32, T]
    sin_t = np.sin(freqs).T.astype(np.float32)
    cos2 = np.concatenate([cos_t] * 4, axis=0)  # [128, T]
    sin2 = np.concatenate([sin_t] * 4, axis=0)
    sgn = np.ones((128, 1), np.float32)
    sgn[0:32] = -1.0
    sgn[64:96] = -1.0
    sinsw = -sgn * sin2
    return _to_dt(cos2), _to_dt(sinsw)


def _tri01():
    j = np.arange(128)[:, None]
    c = np.arange(128)[None, :]
    t = np.where(j <= c, 1.0, 0.0).astype(np.float32)  # [128, 128]
    return _to_dt(np.repeat(t[:, None, :], 2, axis=1))  # [128, 2, 128]


_NC_CACHE = {}
LAST_RESULTS = None  # BassKernelResults of the most recent kernel() call


def kernel(x, wq, wk, wv, wo):
    global LAST_RESULTS
    from concourse.bass_utils import run_bass_kernel_spmd

    x = np.asarray(x, dtype=np.float32)
    wq = np.asarray(wq, dtype=np.float32)
    wk = np.asarray(wk, dtype=np.float32)
    wv = np.asarray(wv, dtype=np.float32)
    wo = np.asarray(wo, dtype=np.float32)

    cos2, sinsw = _rope_tables()
    tri = _tri01()

    in_maps = []
    for core in range(N_CORES):
        b, g = core // 2, core % 2
        gs = slice(G * g, G * g + G)
        in_maps.append(
            {
                "xT": _to_dt(x[b].T),
                "wqT": _to_dt(wq[gs].T),
                "wkT": _to_dt(wk[gs].T),
                "wvT": _to_dt(wv[gs].T),
                "woT": _to_dt(wo[:, gs].T),
                "cos2": cos2,
                "sinsw": sinsw,
                "tri01": tri,
            }
        )

    if "nc" not in _NC_CACHE:
        _NC_CACHE["nc"] = build_kernel()
    nc = _NC_CACHE["nc"]

    res = run_bass_kernel_spmd(nc, in_maps, core_ids=list(range(N_CORES)))
    LAST_RESULTS = res
    outs = [r["out"] for r in res.results]
    full = np.empty((B, T, D), dtype=np.float32)
    for b in range(B):
        full[b] = (
            outs[2 * b].astype(np.float64) + outs[2 * b + 1].astype(np.float64)
        ).astype(np.float32)
    return full


# revision 32
# speedup vs baseline: 1.0285x; 1.0165x over previous
"""Causal self-attention with RoPE on 8 TRN2 NeuronCores — v3.

Problem: B=4, T=2048, D=1024, 16 heads x 64 dims, fp32, causal, RoPE.

Sharding: (batch b, head-group g) -> core b*2+g. Each core computes the
full sequence for 8 heads of one batch plus that group's partial output
projection; the host sums the two partial projections per batch.

v3 changes over v2:
  - fine-grained interleaving: projection / output-projection / normalize
    work is queued as small closures and pumped between attention j-tile
    steps, so the PE stays fed while the ACT engine grinds through exp
    (the attention inner loop is exp-bound at ~1.1us per j-tile vs
    ~0.64us of PE work)
  - reciprocal on ACT (Ln + Exp(-x), same natural_log_exp table set)
    instead of DVE's ~3.3us RECIPROCAL
  - initial weight DMAs split so the first rope matmul only waits for
    x chunk 0 + wq pair 0 (~1.3MB) instead of all weights
"""

import numpy as np
import ml_dtypes

import concourse.bass as bass
import concourse.tile as tile
import concourse.mybir as mybir

F32 = mybir.dt.float32
BF16 = mybir.dt.bfloat16

B, T, D = 4, 2048, 1024
NUM_HEADS, HEAD_DIM = 16, 64
ROPE_THETA = 10000.0

G = 512          # head dims per core (8 heads)
HPC = 8          # heads per core
PAIRS = 4        # pair-tiles (2 heads / 128 partitions)
KT = D // 128    # k-tiles over D
TC = 512         # i-chunk width
NCHUNK = T // TC
TT = T // 128    # t-tiles
N_CORES = 8

DT = BF16


def _split_multi_waits(nc, max_waits=1):
    """This walrus build rejects >1 sync-wait per instruction; spill extras
    onto same-engine NoOps placed just before."""
    counter = [0]
    for func in nc.m.functions:
        for bb in func.blocks:
            insts = bb.instructions
            if not any(
                ins.sync_info is not None and len(ins.sync_info.on_wait) > max_waits
                for ins in insts
            ):
                continue
            new_list = []
            for ins in insts:
                si = ins.sync_info
                if si is None or len(si.on_wait) <= max_waits:
                    new_list.append(ins)
                    continue
                waits = list(si.on_wait)
                spill, keep = waits[:-max_waits], waits[-max_waits:]
                for w in spill:
                    counter[0] += 1
                    new_list.append(
                        mybir.InstNoOp(
                            name=f"waitnop-{counter[0]}",
                            engine=ins.engine,
                            ins=[],
                            outs=[],
                            sync_info=mybir.SyncInfo(on_wait=[w], on_update=[]),
                        )
                    )
                ins.sync_info = mybir.SyncInfo(on_wait=keep, on_update=list(si.on_update))
                new_list.append(ins)
            bb.instructions = new_list


OPTS = {
    "recip": "act",      # act | dve
    "tail_recip": "act",
    "es_bufs": 8,
    "btmp_bufs": 4,
    "attu_bufs": 5,
    "mm_item": 2,        # proj matmuls per work item
}


def build_kernel():
    nc = bass.Bass()

    xT = nc.dram_tensor("xT", [D, T], DT, kind="ExternalInput")
    wqT = nc.dram_tensor("wqT", [D, G], DT, kind="ExternalInput")
    wkT = nc.dram_tensor("wkT", [D, G], DT, kind="ExternalInput")
    wvT = nc.dram_tensor("wvT", [D, G], DT, kind="ExternalInput")
    woT = nc.dram_tensor("woT", [G, D], DT, kind="ExternalInput")
    cos2 = nc.dram_tensor("cos2", [128, T], DT, kind="ExternalInput")
    sinsw = nc.dram_tensor("sinsw", [128, T], DT, kind="ExternalInput")
    tri01 = nc.dram_tensor("tri01", [128, 2, 128], DT, kind="ExternalInput")
    out = nc.dram_tensor("out", [T, D], F32, kind="ExternalOutput")

    LN = mybir.ActivationFunctionType.Ln
    EXP = mybir.ActivationFunctionType.Exp

    with tile.TileContext(nc) as tc:
        with (
            tc.tile_pool(name="const", bufs=1) as cpool,
            tc.tile_pool(name="qk", bufs=1) as qkpool,
            tc.tile_pool(name="vext", bufs=1) as vpool,
            tc.tile_pool(name="attn", bufs=1) as apool,
            tc.tile_pool(name="rope", bufs=2) as rpool,
            tc.tile_pool(name="exps", bufs=8) as epool,
            tc.tile_pool(name="norm", bufs=2) as npool,
            tc.tile_pool(name="outp", bufs=2) as opool,
            tc.tile_pool(name="dramb", bufs=4, space="DRAM") as dpool,
            tc.tile_pool(name="mm", bufs=2, space="PSUM") as mmps,
            tc.tile_pool(name="sp", bufs=2, space="PSUM") as spps,
            tc.tile_pool(name="ap", bufs=1, space="PSUM") as apps,
        ):
            xT_r = xT.rearrange("(k p) t -> p k t", p=128)

            # ---- staged initial DMAs: only what the first rope needs up
            # front; remaining weights stream in behind ----
            xc0 = rpool.tile([128, KT, TC], DT, name="xc0", tag="xc")
            wq_sb = cpool.tile([128, KT, G], DT, name="wq_sb")
            wqT_r = wqT.rearrange("(k p) g -> p k g", p=128)
            nc.sync.dma_start(xc0[:, 0:2, :], xT_r[:, 0:2, bass.ts(0, TC)])
            nc.sync.dma_start(wq_sb[:, 0:2, 0:128], wqT_r[:, 0:2, 0:128])
            nc.sync.dma_start(xc0[:, 2:4, :], xT_r[:, 2:4, bass.ts(0, TC)])
            nc.sync.dma_start(wq_sb[:, 2:4, 0:128], wqT_r[:, 2:4, 0:128])
            nc.sync.dma_start(xc0[:, 4:KT, :], xT_r[:, 4:KT, bass.ts(0, TC)])
            nc.sync.dma_start(wq_sb[:, 4:KT, 0:128], wqT_r[:, 4:KT, 0:128])
            cos_sb = cpool.tile([128, T], DT, name="cos_sb")
            sinsw_sb = cpool.tile([128, T], DT, name="sinsw_sb")
            nc.sync.dma_start(sinsw_sb[:, 0:TC], sinsw[:, 0:TC])
            nc.sync.dma_start(cos_sb[:, 0:TC], cos2[:, 0:TC])
            wk_sb = cpool.tile([128, KT, G], DT, name="wk_sb")
            wkT_r = wkT.rearrange("(k p) g -> p k g", p=128)
            nc.sync.dma_start(wk_sb[:, :, 0:128], wkT_r[:, :, 0:128])
            wv_sb = cpool.tile([128, KT, G], DT, name="wv_sb")
            nc.sync.dma_start(wv_sb[:], wvT.rearrange("(k p) g -> p k g", p=128))
            tri_sb = cpool.tile([128, 2, 128], DT, name="tri_sb")
            nc.sync.dma_start(tri_sb[:], tri01[:])
            wo_sb = cpool.tile([128, PAIRS, D], DT, name="wo_sb")
            ones_sb = cpool.tile([128, 64], DT, name="ones_sb")
            nc.vector.memset(ones_sb[:], 1.0)

            def rest_dmas_a():
                # issued after the pair-0 rope swaps so they don't delay them
                nc.sync.dma_start(wq_sb[:, :, 128:G], wqT_r[:, :, 128:G])
                nc.sync.dma_start(wk_sb[:, :, 128:G], wkT_r[:, :, 128:G])

            def rest_dmas_b():
                nc.sync.dma_start(sinsw_sb[:, TC:T], sinsw[:, TC:T])
                nc.sync.dma_start(cos_sb[:, TC:T], cos2[:, TC:T])

            def rest_dmas_c():
                nc.sync.dma_start(
                    wo_sb[:], woT.rearrange("(k p) d -> p k d", p=128)
                )

            qrot = qkpool.tile([128, PAIRS, T], DT, name="qrot")
            krot = qkpool.tile([128, PAIRS, T], DT, name="krot")
            v_ext = vpool.tile([128, TT, HPC, 65], DT, name="v_ext")
            attnT = apool.tile([128, PAIRS, T], DT, name="attnT")

            pend = {}
            xcs = {0: xc0}

            WORK = []

            def pump(n):
                for _ in range(n):
                    if not WORK:
                        return
                    WORK.pop(0)()

            def drain():
                while WORK:
                    WORK.pop(0)()

            # ---------- projection work items ----------

            def rope_unit_items(c, p, w_sb, rot):
                csl = bass.ts(c, TC)
                st = {}

                def mk_mm(k0, k1):
                    def f():
                        if "ps" not in st:
                            st["ps"] = mmps.tile(
                                [128, TC], F32, name="proj_ps", tag="mmps"
                            )
                        ps = st["ps"]
                        xc = xcs[c]
                        for k in range(k0, k1):
                            nc.tensor.matmul(
                                ps[:],
                                w_sb[:, k, bass.ts(p, 128)],
                                xc[:, k, :],
                                start=(k == 0),
                                stop=(k == KT - 1),
                            )
                    return f

                def dve():
                    ps = st["ps"]
                    u = rpool.tile([128, TC], DT, name="u", tag="u")
                    nc.vector.tensor_mul(u[:], ps[:], sinsw_sb[:, csl])
                    sw = rpool.tile([128, TC], DT, name="sw", tag="sw")
                    for blk in range(4):
                        sc = (blk ^ 1) * 32
                        nc.sync.dma_start(
                            sw[blk * 32 : blk * 32 + 32, :], u[sc : sc + 32, :]
                        )
                    t2 = rpool.tile([128, TC], DT, name="t2", tag="t2")
                    nc.vector.tensor_mul(t2[:], ps[:], cos_sb[:, csl])
                    nc.vector.tensor_add(rot[:, p, csl], t2[:], sw[:])

                step = OPTS["mm_item"]
                items = [mk_mm(k, min(k + step, KT)) for k in range(0, KT, step)]
                items.append(dve)
                return items

            def projv_unit_items(c, tt):
                t = 4 * c + tt
                st = {}

                def mk_mm(k0, k1):
                    def f():
                        if "ps" not in st:
                            st["ps"] = mmps.tile(
                                [128, G], F32, name="v_ps", tag="mmps"
                            )
                        ps = st["ps"]
                        xc = xcs[c]
                        for k in range(k0, k1):
                            nc.tensor.matmul(
                                ps[:],
                                xc[:, k, bass.ts(tt, 128)],
                                wv_sb[:, k, :],
                                start=(k == 0),
                                stop=(k == KT - 1),
                            )
                    return f

                def evac():
                    ps = st["ps"]
                    nc.vector.tensor_copy(
                        v_ext[:, t, :, 0:64],
                        ps[:].rearrange("p (h d) -> p h d", h=HPC),
                    )
                    nc.vector.memset(v_ext[:, t, :, 64:65], 1.0)

                step = OPTS["mm_item"]
                items = [mk_mm(k, min(k + step, KT)) for k in range(0, KT, step)]
                items.append(evac)
                return items

            def proj_items(c):
                items = []

                def xdma():
                    xcn = rpool.tile([128, KT, TC], DT, name="xcn", tag="xc")
                    nc.sync.dma_start(xcn[:], xT_r[:, :, bass.ts(c, TC)])
                    xcs[c] = xcn

                if c not in xcs:
                    items.append(xdma)
                items += rope_unit_items(c, 0, wq_sb, qrot)
                items += rope_unit_items(c, 0, wk_sb, krot)
                if c == 0:
                    items.append(rest_dmas_a)
                for tt in range(4):
                    items += projv_unit_items(c, tt)
                if c == 0:
                    items.append(rest_dmas_b)
                for p in range(1, PAIRS):
                    items += rope_unit_items(c, p, wq_sb, qrot)
                    items += rope_unit_items(c, p, wk_sb, krot)
                if c == 0:
                    items.append(rest_dmas_c)
                return items

            # ---------- output projection items ----------

            def outproj_items(c):
                items = []
                for tt in range(4):
                    t = 4 * c + tt
                    tsl = bass.ts(t, 128)
                    st = {}

                    def mk_dc(dc, st=st, tsl=tsl, t=t):
                        def f():
                            if "ob" not in st:
                                st["ob"] = opool.tile(
                                    [128, D], F32, name="ob", tag="ob"
                                )
                            dsl = bass.ts(dc, 512)
                            ps = mmps.tile([128, 512], F32, name="o_ps", tag="mmps")
                            for p in range(PAIRS):
                                nc.tensor.matmul(
                                    ps[:],
                                    attnT[:, p, tsl],
                                    wo_sb[:, p, dsl],
                                    start=(p == 0),
                                    stop=(p == PAIRS - 1),
                                )
                            nc.vector.tensor_copy(st["ob"][:, dsl], ps[:])
                            nc.sync.dma_start(
                                out[t * 128 : t * 128 + 128, dsl], st["ob"][:, dsl]
                            )
                        return f

                    items += [mk_dc(0), mk_dc(1)]
                return items

            # ---------- softmax normalization ----------

            def recip_rows(den_sb, rden, r0, r1, engine=None):
                if (engine or OPTS["recip"]) == "act":
                    nc.scalar.activation(den_sb[r0:r1, :], den_sb[r0:r1, :], LN)
                    nc.scalar.activation(
                        rden[r0:r1, :], den_sb[r0:r1, :], EXP, scale=-1.0
                    )
                else:
                    with nc.allow_low_precision(reason="bf16 recip rows"):
                        nc.vector.reciprocal(rden[r0:r1, :], den_sb[r0:r1, :])

            def bounce(rden, r0, nrows):
                dscr = dpool.tile([nrows, TC], F32, name="dscr", tag=f"dscr{nrows}")
                nc.sync.dma_start(dscr[:], rden[r0 : r0 + nrows, :])
                rbc = npool.tile(
                    [64, nrows, TC], F32, name="rbc", tag=f"rbc{nrows}", bufs=2
                )
                dsrc = dscr[:]
                nc.sync.dma_start(
                    rbc[:],
                    bass.AP(
                        tensor=dsrc.tensor,
                        offset=dsrc.offset,
                        ap=[[0, 64]] + dsrc.ap,
                    ),
                )
                return rbc

            def norm_muls(c, p, attU, rbc, rcol):
                csl = bass.ts(c, TC)
                nc.vector.tensor_mul(
                    attnT[0:64, p, csl], attU[0:64, 0, :], rbc[:, rcol, :]
                )
                btmp = npool.tile(
                    [64, TC], DT, name="btmp", tag="btmp", bufs=OPTS["btmp_bufs"]
                )
                nc.vector.tensor_mul(btmp[:], attU[0:64, 1, :], rbc[:, rcol + 1, :])
                nc.sync.dma_start(attnT[64:128, p, csl], btmp[:])

            def normalize_items(c):
                items = []

                def recip():
                    attUs, den_sb, rden = pend[c]
                    recip_rows(den_sb, rden, 0, 8)

                items.append(recip)
                for p in range(PAIRS):

                    def mk_np(p=p):
                        def f():
                            attUs, den_sb, rden = pend[c]
                            rbc = bounce(rden, 2 * p, 2)
                            norm_muls(c, p, attUs[p], rbc, 0)
                        return f

                    items.append(mk_np())
                items.append(lambda: pend.pop(c))
                return items

            # ---------- attention ----------

            def attn_pair(c, p, den_sb, dr0, steps_left, last=False):
                atts = apps.tile([128, 2, TC], F32, name="att_ps", tag="apps")
                njt = 4 * c + 4

                def av(jt, es, off, fd):
                    for hh in range(2):
                        nc.tensor.matmul(
                            atts[0:65, hh, off : off + fd],
                            v_ext[:, jt, 2 * p + hh, :],
                            es[:, hh, 0:fd],
                            start=(jt == 0),
                            stop=(jt == njt - 1),
                        )

                for jt in range(njt):
                    m = jt - 4 * c
                    sAB = spps.tile([128, 2, TC], F32, name="s_ps", tag="spps")
                    soff = 128 * m if m > 0 else 0
                    for hh in range(2):
                        hsl = slice(64 * hh, 64 * hh + 64)
                        nc.tensor.matmul(
                            sAB[:, hh, soff:TC],
                            krot[hsl, p, bass.ts(jt, 128)],
                            qrot[hsl, p, c * TC + soff : (c + 1) * TC],
                            start=True,
                            stop=True,
                            tile_position=(64 * hh, 0),
                        )
                    off = 128 * m if m > 0 else 0
                    fd = TC - off
                    es = epool.tile(
                        [128, 2, TC], DT, name="es", tag="es", bufs=OPTS["es_bufs"]
                    )
                    nc.scalar.activation(
                        es[:, :, 0:fd],
                        sAB[:, :, off : off + fd],
                        EXP,
                        scale=0.125,
                    )
                    if m >= 0:
                        nc.vector.tensor_mul(
                            es[:, :, 0:128], es[:, :, 0:128], tri_sb[:]
                        )
                    av(jt, es, off, fd)
                    # feed the PE between exp-bound steps
                    sl = steps_left[0]
                    if WORK and sl > 0:
                        r = -(-len(WORK) // sl)
                        pump(min(r, 6))
                    steps_left[0] -= 1
                attU = npool.tile(
                    [128, 2, TC], F32, name="attU", tag="attU",
                    bufs=OPTS["attu_bufs"],
                )
                if last:
                    # sliced tail evacuates per t-slice itself
                    return attU, atts
                # per-head evacuation: the next pair's first AV (start=True)
                # only waits for its own bank's reader
                for hh in range(2):
                    nc.vector.tensor_copy(attU[0:65, hh, :], atts[0:65, hh, :])
                    nc.sync.dma_start(
                        den_sb[dr0 + hh : dr0 + hh + 1, :], attU[64:65, hh, :]
                    )
                return attU, atts

            def tail_norm_pair(c, p, attU, atts, rdrow, sliced):
                """Last-chunk normalize without DRAM bounces: reciprocal on ACT
                straight from the AV psum's denominator row, broadcast across
                partitions with a K=1 matmul, multiply. When `sliced`, process
                128-wide t-slices and chase each with its output projection to
                keep the PE warm through the tail."""
                c0 = c * TC
                nslice = 4 if sliced else 1
                w = TC // nslice
                rbc = spps.tile([128, 2, TC], F32, name="rbc_ps", tag="spps")
                for s in range(nslice):
                    ssl = slice(s * w, s * w + w)
                    for hh in range(2):
                        nc.vector.tensor_copy(
                            attU[0:65, hh, ssl], atts[0:65, hh, ssl]
                        )
                    nc.scalar.activation(
                        attU[64:65, :, ssl], attU[64:65, :, ssl], LN
                    )
                    nc.scalar.activation(
                        rdrow[64:65, :, ssl], attU[64:65, :, ssl], EXP, scale=-1.0
                    )
                    for hh in range(2):
                        nc.tensor.matmul(
                            rbc[0:64, hh, ssl],
                            ones_sb[64:65, :],
                            rdrow[64:65, hh, ssl],
                            start=True,
                            stop=True,
                        )
                    btmp = npool.tile(
                        [64, TC], DT, name="btmp", tag="btmp",
                        bufs=OPTS["btmp_bufs"],
                    )
                    nc.vector.tensor_mul(
                        btmp[:, ssl], attU[0:64, 1, ssl], rbc[0:64, 1, ssl]
                    )
                    nc.sync.dma_start(
                        attnT[64:128, p, c0 + s * w : c0 + (s + 1) * w],
                        btmp[:, ssl],
                    )
                    nc.vector.tensor_mul(
                        attnT[0:64, p, c0 + s * w : c0 + (s + 1) * w],
                        attU[0:64, 0, ssl],
                        rbc[0:64, 0, ssl],
                    )
                    if sliced and s > 0:
                        # project the PREVIOUS slice's t-tile: its btmp DMA
                        # completed while this slice's recip ran
                        outproj_tile(4 * c + s - 1)
                if sliced:
                    outproj_tile(4 * c + nslice - 1)

            def outproj_tile(t):
                tsl = bass.ts(t, 128)
                ob = opool.tile([128, D], F32, name="ob", tag="ob")
                for dc in range(2):
                    dsl = bass.ts(dc, 512)
                    ps = mmps.tile([128, 512], F32, name="o_ps", tag="mmps")
                    for pp in range(PAIRS):
                        nc.tensor.matmul(
                            ps[:],
                            attnT[:, pp, tsl],
                            wo_sb[:, pp, dsl],
                            start=(pp == 0),
                            stop=(pp == PAIRS - 1),
                        )
                    nc.vector.tensor_copy(ob[:, dsl], ps[:])
                    nc.sync.dma_start(
                        out[t * 128 : t * 128 + 128, dsl], ob[:, dsl]
                    )

            def attn_chunk(c):
                last = c == NCHUNK - 1
                den_sb = npool.tile([128, TC], F32, name="den_sb", tag="den")
                rden = npool.tile([128, TC], F32, name="rden", tag="rden")
                attUs = []
                steps_left = [PAIRS * (4 * c + 4)]
                for p in range(PAIRS):
                    tail_sliced = last and p == PAIRS - 1
                    dr0 = 32 * p if last else 2 * p
                    attU, atts = attn_pair(
                        c, p, den_sb, dr0, steps_left, last=tail_sliced
                    )
                    if tail_sliced:
                        rdrow = npool.tile(
                            [65, 2, TC], DT, name="rdrow", tag="rdrow", bufs=1
                        )
                        tail_norm_pair(c, p, attU, atts, rdrow, sliced=True)
                    elif last:
                        # earlier tail pairs: normalize per-pair off the PE
                        # path (den rows -> ACT recip -> DRAM-bounce bcast)
                        recip_rows(den_sb, rden, 32 * p, 32 * p + 2)
                        rbc = bounce(rden, 32 * p, 2)
                        norm_muls(c, p, attU, rbc, 0)
                    else:
                        attUs.append(attU)
                if not last:
                    pend[c] = (attUs, den_sb, rden)

            # ---------- main schedule ----------

            for it in proj_items(0):
                it()
            for c in range(NCHUNK):
                if c > 0:
                    WORK.extend(normalize_items(c - 1))
                if c + 1 < NCHUNK:
                    WORK.extend(proj_items(c + 1))
                # ALL output projections are deferred into the last chunk's
                # attention window, which is otherwise exp-bound with an
                # idle PE; earlier windows are already PE-bound
                if c == NCHUNK - 1:
                    for cc in range(NCHUNK - 1):
                        WORK.extend(outproj_items(cc))
                attn_chunk(c)
                drain()

    _split_multi_waits(nc)
    return nc


def _to_dt(x):
    return np.ascontiguousarray(x).astype(ml_dtypes.bfloat16)


def _rope_tables():
    inv_freq = 1.0 / ROPE_THETA ** (np.arange(0, HEAD_DIM, 2, dtype=np.float64) / HEAD_DIM)
    freqs = np.outer(np.arange(T, dtype=np.float64), inv_freq)  # [T, 32]
    cos_t = np.cos(freqs).T.astype(np.float32)  # [32, T]
    sin_t = np.sin(freqs).T.astype(np.float32)
    cos2 = np.concatenate([cos_t] * 4, axis=0)  # [128, T]
    sin2 = np.concatenate([sin_t] * 4, axis=0)
    sgn = np.ones((128, 1), np.float32)
    sgn[0:32] = -1.0
    sgn[64:96] = -1.0
    sinsw = -sgn * sin2
    return _to_dt(cos2), _to_dt(sinsw)


def _tri01():
    j = np.arange(128)[:, None]
    c = np.arange(128)[None, :]
    t = np.where(j <= c, 1.0, 0.0).astype(np.float32)  # [128, 128]
    return _to_dt(np.repeat(t[:, None, :], 2, axis=1))  # [128, 2, 128]


_NC_CACHE = {}
LAST_RESULTS = None  # BassKernelResults of the most recent kernel() call


def kernel(x, wq, wk, wv, wo):
    global LAST_RESULTS
    from concourse.bass_utils import run_bass_kernel_spmd

    x = np.asarray(x, dtype=np.float32)
    wq = np.asarray(wq, dtype=np.float32)
    wk = np.asarray(wk, dtype=np.float32)
    wv = np.asarray(wv, dtype=np.float32)
    wo = np.asarray(wo, dtype=np.float32)

    cos2, sinsw = _rope_tables()
    tri = _tri01()

    in_maps = []
    for core in range(N_CORES):
        b, g = core // 2, core % 2
        gs = slice(G * g, G * g + G)
        in_maps.append(
            {
                "xT": _to_dt(x[b].T),
                "wqT": _to_dt(wq[gs].T),
                "wkT": _to_dt(wk[gs].T),
                "wvT": _to_dt(wv[gs].T),
                "woT": _to_dt(wo[:, gs].T),
                "cos2": cos2,
                "sinsw": sinsw,
                "tri01": tri,
            }
        )

    if "nc" not in _NC_CACHE:
        _NC_CACHE["nc"] = build_kernel()
    nc = _NC_CACHE["nc"]

    res = run_bass_kernel_spmd(nc, in_maps, core_ids=list(range(N_CORES)))
    LAST_RESULTS = res
    outs = [r["out"] for r in res.results]
    full = np.empty((B, T, D), dtype=np.float32)
    for b in range(B):
        full[b] = (
            outs[2 * b].astype(np.float64) + outs[2 * b + 1].astype(np.float64)
        ).astype(np.float32)
    return full
